# revision 6
# baseline (speedup 1.0000x reference)
"""Trainium2 Bass kernel for nn_BBTT_GNN (gnn_message_passing).

Strategy (pure data parallel, 8 cores, batch-sharded):
  - Feature-major on-chip layout: features on SBUF partitions, events on the
    free dim.  4 event "groups" live in the 4 32-partition blocks, so every
    matmul processes 4*512 = 2048 events per N=512 instruction.
  - All per-node 8x8 layers are packed into block-diagonal 128x128 lhsT
    weights (host-prepped); aggregation (adj@n/deg) is folded into a matmul.
  - Graph dedup: ADJ rows/cols 2==3 and 4==5, so after the first aggregation
    only 4 distinct node states exist -> layers 1..3 use 32 feats/group.
  - leaky_relu(0.01) + bias fused into one ScalarE Prelu pass per layer.
  - log_softmax(2) = -(relu(d)+log1p(exp(-|d|))) with d = +/-(v1-v0), done
    with Abs/Exp/Ln/Relu ACT passes once per supertile.
  - PE transposes (regular matmuls vs identity) move events<->features at
    entry/exit so all DMAs are large and contiguous.

Event mapping inside a supertile of 8192 events:
  event = st*8192 + 64*p + eps,  eps = 16*m + 4*c + g
  (p: transpose column 0..127, m: MM-pass 0..3, c: 128-col chunk 0..3,
   g: partition group 0..3).  This makes every DMA (x, o loads and y store)
  contiguous per partition.
"""

import os
import sys
from contextlib import ExitStack

for _p in ("/opt/trn_rl_repo", "/root/.axon_site/_ro/trn_rl_repo"):
    if os.path.isdir(_p) and _p not in sys.path:
        sys.path.insert(0, _p)

import numpy as np

import concourse.bass as bass
import concourse.tile as tile
from concourse import bacc, mybir
from concourse.bass_utils import run_bass_kernel_spmd

F32 = mybir.dt.float32
AF = mybir.ActivationFunctionType

NCORES = 8
B_TOTAL = 1048576
BC = B_TOTAL // NCORES          # 131072 events per core
ST_EV = 8192                    # events per supertile
NST = BC // ST_EV               # 16 supertiles
NPASS = 4                       # MM passes per supertile (2048 events each)
NCOL = 512                      # matmul free dim

ADJ = np.array(
    [[1, 1, 1, 1, 0, 0], [1, 1, 0, 0, 1, 1], [1, 0, 1, 1, 0, 0],
     [1, 0, 1, 1, 0, 0], [0, 1, 0, 0, 1, 1], [0, 1, 0, 0, 1, 1]],
    dtype=np.float64)
DEG = ADJ.sum(axis=1)           # [4,4,3,3,3,3]
DNODES = [0, 1, 2, 4]           # distinct nodes after first aggregation
MULT = [1, 1, 2, 2]
ALPHA = 0.01


# --------------------------------------------------------------------------
# Host-side constant packing
# --------------------------------------------------------------------------

class _Pack:
    def __init__(self):
        self.cols = 0
        self.slices = {}
        self.mats = []

    def add(self, name, mat):
        mat = np.asarray(mat, dtype=np.float32)
        assert mat.shape[0] <= 128
        m = np.zeros((128, mat.shape[1]), np.float32)
        m[: mat.shape[0]] = mat
        self.slices[name] = (self.cols, mat.shape[1])
        self.mats.append(m)
        self.cols += mat.shape[1]

    def array(self):
        return np.concatenate(self.mats, axis=1)


def _prep_consts(hl0_W, hl0_b, hl_W, hl_b, g0_W, g0_b, gp_W, gp_b,
                 gf_W, gf_b, fc0_W, fc0_b, fc1_W, fc1_b):
    f64 = np.float64
    hl0_W, hl_W, g0_W, gp_W, gf_W, fc0_W, fc1_W = [
        np.asarray(a, f64) for a in (hl0_W, hl_W, g0_W, gp_W, gf_W, fc0_W, fc1_W)]

    A06 = np.zeros((4, 6), f64)   # layer0 agg: 6 nodes -> 4 distinct
    A44 = np.zeros((4, 4), f64)   # layers1-3 agg: 4 -> 4 with multiplicity
    for i, di in enumerate(DNODES):
        for j in range(6):
            A06[i, j] = ADJ[di, j] / DEG[di]
        for j, dj in enumerate(DNODES):
            A44[i, j] = ADJ[di, dj] * MULT[j] / DEG[di]

    wp = _Pack()
    wp.add("ident", np.eye(128, dtype=np.float32))

    negI = np.zeros((128, 8), f64)
    for m in range(4):
        for k in range(8):
            negI[32 * m + k, k] = -1.0
    wp.add("negI", negI)

    # L0 proj0 (3->8 per node), halves A (groups 0,1) and B (groups 2,3)
    for half, gs in (("A", (0, 1)), ("B", (2, 3))):
        w = np.zeros((128, 112), f64)
        for gi, g in enumerate(gs):
            for n in range(6):
                for f in range(3):
                    for h in range(8):
                        w[32 * g + 3 * n + f, 64 * gi + 8 * n + h] = g0_W[f, h]
        wp.add(f"l0p0{half}", w)

    # L0 proj1/proj2 (8->8 per node), same matrix for both halves
    for pi, k in (("l0p1", 0), ("l0p2", 1)):
        w = np.zeros((112, 112), f64)
        for gi in range(2):
            for n in range(6):
                w[64 * gi + 8 * n:64 * gi + 8 * n + 8,
                  64 * gi + 8 * n:64 * gi + 8 * n + 8] = gp_W[k]
        wp.add(pi, w)

    # L0 agg: [6 nodes x 8] -> [4 distinct x 8], per half (out base differs)
    w = np.zeros((112, 64), f64)
    for gi in range(2):
        for ip in range(4):
            for j in range(6):
                for f in range(8):
                    w[64 * gi + 8 * j + f, 32 * gi + 8 * ip + f] = A06[ip, j]
    wp.add("l0agg", w)

    # L1..3 proj + agg, 4-group block diagonal [128,128]
    for l in range(1, 4):
        for p in range(3):
            k = 2 + 3 * (l - 1) + p
            w = np.zeros((128, 128), f64)
            for g in range(4):
                for n in range(4):
                    w[32 * g + 8 * n:32 * g + 8 * n + 8,
                      32 * g + 8 * n:32 * g + 8 * n + 8] = gp_W[k]
            wp.add(f"l{l}p{p}", w)
        w = np.zeros((128, 128), f64)
        for g in range(4):
            for ip in range(4):
                for j in range(4):
                    for f in range(8):
                        w[32 * g + 8 * j + f, 32 * g + 8 * ip + f] = A44[ip, j]
        wp.add(f"l{l}agg", w)

    # aux branch
    w = np.zeros((128, 32), f64)
    for g in range(4):
        for q in range(5):
            for h in range(8):
                w[32 * g + 18 + q, 8 * g + h] = hl0_W[q, h]
    wp.add("hl0", w)
    for i in range(3):
        w = np.zeros((32, 32), f64)
        for g in range(4):
            w[8 * g:8 * g + 8, 8 * g:8 * g + 8] = hl_W[i]
        wp.add(f"hl{i + 1}", w)

    # gf (48->8) folded over duplicate nodes
    gfW4 = [gf_W[0:8], gf_W[8:16], gf_W[16:24] + gf_W[24:32],
            gf_W[32:40] + gf_W[40:48]]
    w = np.zeros((128, 32), f64)
    for g in range(4):
        for n in range(4):
            for f in range(8):
                for h in range(8):
                    w[32 * g + 8 * n + f, 8 * g + h] = gfW4[n][f, h]
    wp.add("gf", w)

    # fc0: rows 0..31 = h (fc0_W rows 8..15), rows 32..63 = g (rows 0..7)
    w = np.zeros((64, 64), f64)
    for g in range(4):
        for j in range(8):
            for m in range(16):
                w[8 * g + j, 16 * g + m] = fc0_W[8 + j, m]
                w[32 + 8 * g + j, 16 * g + m] = fc0_W[j, m]
    wp.add("fc0", w)

    # fc1 folded with the log_softmax difference matrix
    dw = fc1_W[:, 1] - fc1_W[:, 0]
    w = np.zeros((64, 8), f64)
    for g in range(4):
        for m in range(16):
            w[16 * g + m, 2 * g] = dw[m]
            w[16 * g + m, 2 * g + 1] = -dw[m]
    wp.add("fc1d", w)

    # biases
    bp = _Pack()
    b = np.zeros((112, 1), f64)
    for gi in range(2):
        for n in range(6):
            b[64 * gi + 8 * n:64 * gi + 8 * n + 8, 0] = g0_b
    bp.add("l0p0", b)
    for nm, k in (("l0p1", 0), ("l0p2", 1)):
        b = np.zeros((112, 1), f64)
        for gi in range(2):
            for n in range(6):
                b[64 * gi + 8 * n:64 * gi + 8 * n + 8, 0] = gp_b[k]
        bp.add(nm, b)
    for l in range(1, 4):
        for p in range(3):
            k = 2 + 3 * (l - 1) + p
            b = np.zeros((128, 1), f64)
            for g in range(4):
                for n in range(4):
                    b[32 * g + 8 * n:32 * g + 8 * n + 8, 0] = gp_b[k]
            bp.add(f"l{l}p{p}", b)
    b = np.zeros((32, 1), f64)
    for g in range(4):
        b[8 * g:8 * g + 8, 0] = hl0_b
    bp.add("hl0", b)
    for i in range(2):
        b = np.zeros((32, 1), f64)
        for g in range(4):
            b[8 * g:8 * g + 8, 0] = hl_b[i]
        bp.add(f"hl{i + 1}", b)
    b = np.zeros((64, 1), f64)
    for g in range(4):
        b[8 * g:8 * g + 8, 0] = hl_b[2]
        b[32 + 8 * g:32 + 8 * g + 8, 0] = gf_b
    bp.add("Z", b)
    b = np.zeros((64, 1), f64)
    for g in range(4):
        b[16 * g:16 * g + 16, 0] = fc0_b
    bp.add("fc0", b)
    delta = float(fc1_b[1]) - float(fc1_b[0])
    b = np.zeros((104, 1), f64)
    for m in range(4):
        for g in range(4):
            b[32 * m + 2 * g, 0] = delta
            b[32 * m + 2 * g + 1, 0] = -delta
    bp.add("head", b)

    return wp, bp


# --------------------------------------------------------------------------
# Device program
# --------------------------------------------------------------------------

def _build_program(wslices, bslices, wcols, bcols, nst=NST):
    nc = bacc.Bacc("TRN2", target_bir_lowering=False, debug=False,
                   enable_asserts=False, num_devices=NCORES)
    bc = nst * ST_EV
    x_d = nc.dram_tensor("x", [bc, 18], F32, kind="ExternalInput")
    o_d = nc.dram_tensor("o", [bc, 5], F32, kind="ExternalInput")
    w_d = nc.dram_tensor("wpack", [128, wcols], F32, kind="ExternalInput")
    b_d = nc.dram_tensor("bpack", [128, bcols], F32, kind="ExternalInput")
    y_d = nc.dram_tensor("y", [bc, 2], F32, kind="ExternalOutput")

    xv = x_d.ap().rearrange("(s p e) f -> s p (e f)", s=nst, p=128, e=64)
    ov = o_d.ap().rearrange("(s p e) f -> s p (e f)", s=nst, p=128, e=64)
    yv = y_d.ap().rearrange("(s p e) j -> s p (e j)", s=nst, p=128, e=64)

    with TileCtx(nc) as (tc, ctx):
        wpool = ctx.enter_context(tc.tile_pool(name="wts", bufs=1))
        ldpool = ctx.enter_context(tc.tile_pool(name="loads", bufs=2))
        stgpool = ctx.enter_context(tc.tile_pool(name="stg", bufs=2))
        acts = ctx.enter_context(tc.tile_pool(name="acts", bufs=10))
        opool = ctx.enter_context(tc.tile_pool(name="outs", bufs=2))
        psum = ctx.enter_context(
            tc.tile_pool(name="ps", bufs=4, space="PSUM"))
        dpool = ctx.enter_context(
            tc.tile_pool(name="dps", bufs=3, space="PSUM"))

        wt = wpool.tile([128, wcols], F32)
        nc.sync.dma_start(out=wt[:], in_=w_d.ap())
        bt = wpool.tile([128, bcols], F32)
        nc.sync.dma_start(out=bt[:], in_=b_d.ap())

        def W(name, rows):
            c0, cw = wslices[name]
            return wt[0:rows, c0:c0 + cw]

        def WB(name, r0, rows):
            c0, cw = wslices[name]
            return wt[r0:r0 + rows, c0:c0 + cw]

        def Bias(name, rows):
            c0, cw = bslices[name]
            return bt[0:rows, c0:c0 + 1]

        def prelu(dst, src, bias):
            nc.scalar.activation(dst, src, AF.Prelu, bias=bias, scale=1.0,
                                 alpha=ALPHA)

        for st in range(nst):
            xl = ldpool.tile([128, 64 * 18], F32, tag="xl")
            nc.sync.dma_start(out=xl[:], in_=xv[st])
            olt = ldpool.tile([128, 64 * 5], F32, tag="ol")
            nc.sync.dma_start(out=olt[:], in_=ov[st])

            # repack into 32-col event slots (cols 23..31 zeroed)
            stg = stgpool.tile([128, 64 * 32], F32, tag="stg")
            s3 = stg[:].rearrange("p (e f) -> p e f", f=32)
            nc.vector.memset(s3[:, :, 23:32], 0.0)
            nc.vector.tensor_copy(
                s3[:, :, 0:18],
                xl[:].rearrange("p (e f) -> p e f", f=18))
            nc.vector.tensor_copy(
                s3[:, :, 18:23],
                olt[:].rearrange("p (e f) -> p e f", f=5))

            D_ps = dpool.tile([128, NCOL], F32, tag="dps")
            for m in range(NPASS):
                # ---- transpose 16 chunks into feature-major psum ----
                T_in = psum.tile([128, NCOL], F32, tag="ps")
                for c in range(4):
                    for g in range(4):
                        eps = 16 * m + 4 * c + g
                        nc.tensor.matmul(
                            T_in[32 * g:32 * g + 32, 128 * c:128 * c + 128],
                            stg[:, 32 * eps:32 * eps + 32],
                            wt[0:128, 0:128],  # identity
                            start=True, stop=True,
                            tile_position=(0, 32 * g))
                in0 = acts.tile([128, NCOL], F32, tag="act")
                nc.vector.tensor_copy(in0[:], T_in[:])

                # ---- GNN layer 0 (6 nodes, two halves of 2 groups) ----
                shalves = []
                for half, wname in ((0, "l0p0A"), (1, "l0p0B")):
                    p0 = psum.tile([112, NCOL], F32, tag="ps")
                    nc.tensor.matmul(p0[:], W(wname, 119), in0[0:119, :],
                                     start=True, stop=True)
                    s0 = acts.tile([112, NCOL], F32, tag="act")
                    prelu(s0[:], p0[:], Bias("l0p0", 112))
                    p1 = psum.tile([112, NCOL], F32, tag="ps")
                    nc.tensor.matmul(p1[:], W("l0p1", 112), s0[:],
                                     start=True, stop=True)
                    s1 = acts.tile([112, NCOL], F32, tag="act")
                    prelu(s1[:], p1[:], Bias("l0p1", 112))
                    p2 = psum.tile([112, NCOL], F32, tag="ps")
                    nc.tensor.matmul(p2[:], W("l0p2", 112), s1[:],
                                     start=True, stop=True)
                    s2 = acts.tile([112, NCOL], F32, tag="act")
                    prelu(s2[:], p2[:], Bias("l0p2", 112))
                    shalves.append(s2)
                pagg = psum.tile([128, NCOL], F32, tag="ps")
                nc.tensor.matmul(pagg[0:64, :], W("l0agg", 112),
                                 shalves[0][:], start=True, stop=True)
                nc.tensor.matmul(pagg[64:128, :], W("l0agg", 112),
                                 shalves[1][:], start=True, stop=True)
                G = acts.tile([128, NCOL], F32, tag="act")
                prelu(G[:], pagg[:], 0.0)

                # ---- GNN layers 1..3 (4 distinct nodes, 4 groups) ----
                for l in range(1, 4):
                    for p in range(3):
                        pp = psum.tile([128, NCOL], F32, tag="ps")
                        nc.tensor.matmul(pp[:], W(f"l{l}p{p}", 128), G[:],
                                         start=True, stop=True)
                        Gn = acts.tile([128, NCOL], F32, tag="act")
                        prelu(Gn[:], pp[:], Bias(f"l{l}p{p}", 128))
                        G = Gn
                    pa = psum.tile([128, NCOL], F32, tag="ps")
                    nc.tensor.matmul(pa[:], W(f"l{l}agg", 128), G[:],
                                     start=True, stop=True)
                    G = acts.tile([128, NCOL], F32, tag="act")
                    prelu(G[:], pa[:], 0.0)

                # ---- aux branch ----
                ph = psum.tile([32, NCOL], F32, tag="ps")
                nc.tensor.matmul(ph[:], W("hl0", 119), in0[0:119, :],
                                 start=True, stop=True)
                h = acts.tile([32, NCOL], F32, tag="act")
                prelu(h[:], ph[:], Bias("hl0", 32))
                for i in (1, 2):
                    ph = psum.tile([32, NCOL], F32, tag="ps")
                    nc.tensor.matmul(ph[:], W(f"hl{i}", 32), h[:],
                                     start=True, stop=True)
                    hn = acts.tile([32, NCOL], F32, tag="act")
                    prelu(hn[:], ph[:], Bias(f"hl{i}", 32))
                    h = hn

                # ---- heads: Z = [hl3 | gf], fc0, fc1d ----
                Z = psum.tile([64, NCOL], F32, tag="ps")
                nc.tensor.matmul(Z[0:32, :], W("hl3", 32), h[:],
                                 start=True, stop=True)
                nc.tensor.matmul(Z[32:64, :], W("gf", 128), G[:],
                                 start=True, stop=True)
                zin = acts.tile([64, NCOL], F32, tag="act")
                prelu(zin[:], Z[:], Bias("Z", 64))
                pz = psum.tile([64, NCOL], F32, tag="ps")
                nc.tensor.matmul(pz[:], W("fc0", 64), zin[:],
                                 start=True, stop=True)
                z2 = acts.tile([64, NCOL], F32, tag="act")
                prelu(z2[:], pz[:], Bias("fc0", 64))
                nc.tensor.matmul(D_ps[32 * m:32 * m + 8, :],
                                 W("fc1d", 64), z2[:], start=True, stop=True,
                                 tile_position=(0, 32 * m))

            # ---- per-supertile head: -softplus, transpose out, store ----
            bh = Bias("head", 104)
            ab = acts.tile([104, NCOL], F32, tag="act")
            nc.scalar.activation(ab[:], D_ps[0:104, :], AF.Abs, bias=bh)
            ex = acts.tile([104, NCOL], F32, tag="act")
            nc.scalar.activation(ex[:], ab[:], AF.Exp, scale=-1.0)
            lnt = acts.tile([104, NCOL], F32, tag="act")
            nc.scalar.activation(lnt[:], ex[:], AF.Ln, bias=1.0)
            ret = acts.tile([104, NCOL], F32, tag="act")
            nc.scalar.activation(ret[:], D_ps[0:104, :], AF.Relu, bias=bh)
            sm = acts.tile([104, NCOL], F32, tag="act")
            nc.vector.tensor_add(sm[:], lnt[:], ret[:])

            OutPs = dpool.tile([128, 128], F32, tag="dps")
            for m in range(NPASS):
                for c in range(4):
                    nc.tensor.matmul(
                        OutPs[:, 32 * m + 8 * c:32 * m + 8 * c + 8],
                        sm[32 * m:32 * m + 8, 128 * c:128 * c + 128],
                        WB("negI", 32 * m, 8),
                        start=True, stop=True,
                        tile_position=(32 * m, 0))
            outsb = opool.tile([128, 128], F32, tag="osb")
            nc.vector.tensor_copy(outsb[:], OutPs[:])
            nc.sync.dma_start(out=yv[st], in_=outsb[:])

    nc.compile()
    return nc


class TileCtx:
    """TileContext + ExitStack in one `with`."""

    def __init__(self, nc):
        self.nc = nc

    def __enter__(self):
        self.ctx = ExitStack()
        self.tc = tile.TileContext(self.nc)
        self.tc.__enter__()
        return self.tc, self.ctx

    def __exit__(self, *a):
        self.ctx.close()
        return self.tc.__exit__(*a)


_CACHE = {}


def _get_program(wslices_key, wcols, bcols, nst, wslices, bslices):
    key = (wcols, bcols, nst)
    if key not in _CACHE:
        _CACHE[key] = _build_program(wslices, bslices, wcols, bcols, nst)
    return _CACHE[key]


def kernel(x, o, hl0_W, hl0_b, hl_W, hl_b, g0_W, g0_b, gp_W, gp_b,
           gf_W, gf_b, fc0_W, fc0_b, fc1_W, fc1_b, _nst=NST, _trace=False,
           _tmpdir=None):
    wp, bp = _prep_consts(hl0_W, hl0_b, hl_W, hl_b, g0_W, g0_b, gp_W, gp_b,
                          gf_W, gf_b, fc0_W, fc0_b, fc1_W, fc1_b)
    wpack = wp.array()
    bpack = bp.array()
    nc = _get_program(None, wpack.shape[1], bpack.shape[1], _nst,
                      wp.slices, bp.slices)

    bc = _nst * ST_EV
    ncores_used = min(NCORES, (np.asarray(x).shape[0] + bc - 1) // bc)
    x = np.ascontiguousarray(np.asarray(x, np.float32)).reshape(-1, 18)
    o = np.ascontiguousarray(np.asarray(o, np.float32)).reshape(-1, 5)

    in_maps = []
    for c in range(ncores_used):
        in_maps.append({
            "x": x[c * bc:(c + 1) * bc],
            "o": o[c * bc:(c + 1) * bc],
            "wpack": wpack,
            "bpack": bpack,
        })
    res = run_bass_kernel_spmd(nc, in_maps, list(range(ncores_used)),
                               trace=_trace, tmpdir=_tmpdir)
    ys = [res.results[c]["y"] for c in range(ncores_used)]
    out = np.concatenate(ys, axis=0)
    if _trace:
        kernel._last_exec_ns = res.exec_time_ns
    return out


kernel._last_exec_ns = None


# revision 7
# speedup vs baseline: 1.9789x; 1.9789x over previous
"""Trainium2 Bass kernel for nn_BBTT_GNN (gnn_message_passing).

Strategy (pure data parallel, 8 cores, batch-sharded):
  - Feature-major on-chip layout: features on SBUF partitions, events on the
    free dim.  4 event "groups" live in the 4 32-partition blocks, so every
    matmul processes 4*512 = 2048 events per N=512 instruction.
  - All per-node 8x8 layers are packed into block-diagonal 128x128 lhsT
    weights (host-prepped); aggregation (adj@n/deg) is folded into a matmul.
  - Graph dedup: ADJ rows/cols 2==3 and 4==5, so after the first aggregation
    only 4 distinct node states exist -> layers 1..3 use 32 feats/group.
  - leaky_relu(0.01) + bias fused into one ScalarE Prelu pass per layer.
  - log_softmax(2) = -(relu(d)+log1p(exp(-|d|))) with d = +/-(v1-v0), done
    with Abs/Exp/Ln/Relu ACT passes once per supertile.
  - PE transposes (regular matmuls vs identity) move events<->features at
    entry/exit so all DMAs are large and contiguous.

Event mapping inside a supertile of 8192 events:
  event = st*8192 + 64*p + eps,  eps = 16*m + 4*c + g
  (p: transpose column 0..127, m: MM-pass 0..3, c: 128-col chunk 0..3,
   g: partition group 0..3).  This makes every DMA (x, o loads and y store)
  contiguous per partition.
"""

import os
import sys
from contextlib import ExitStack

for _p in ("/opt/trn_rl_repo", "/root/.axon_site/_ro/trn_rl_repo"):
    if os.path.isdir(_p) and _p not in sys.path:
        sys.path.insert(0, _p)

import numpy as np

import concourse.bass as bass
import concourse.tile as tile
from concourse import bacc, mybir
from concourse.bass_utils import run_bass_kernel_spmd

F32 = mybir.dt.float32
F16 = mybir.dt.float16
AF = mybir.ActivationFunctionType
ACT_DT = F16          # dtype for matmul operands (weights/activations)
ACT_NP = np.float16

NCORES = 8
B_TOTAL = 1048576
BC = B_TOTAL // NCORES          # 131072 events per core
ST_EV = 8192                    # events per supertile
NST = BC // ST_EV               # 16 supertiles
NPASS = 4                       # MM passes per supertile (2048 events each)
NCOL = 512                      # matmul free dim

ADJ = np.array(
    [[1, 1, 1, 1, 0, 0], [1, 1, 0, 0, 1, 1], [1, 0, 1, 1, 0, 0],
     [1, 0, 1, 1, 0, 0], [0, 1, 0, 0, 1, 1], [0, 1, 0, 0, 1, 1]],
    dtype=np.float64)
DEG = ADJ.sum(axis=1)           # [4,4,3,3,3,3]
DNODES = [0, 1, 2, 4]           # distinct nodes after first aggregation
MULT = [1, 1, 2, 2]
ALPHA = 0.01


# --------------------------------------------------------------------------
# Host-side constant packing
# --------------------------------------------------------------------------

class _Pack:
    def __init__(self, np_dtype=np.float32):
        self.np_dtype = np_dtype
        self.cols = 0
        self.slices = {}
        self.mats = []

    def add(self, name, mat):
        mat = np.asarray(mat, dtype=self.np_dtype)
        assert mat.shape[0] <= 128
        m = np.zeros((128, mat.shape[1]), self.np_dtype)
        m[: mat.shape[0]] = mat
        self.slices[name] = (self.cols, mat.shape[1])
        self.mats.append(m)
        self.cols += mat.shape[1]

    def array(self):
        return np.concatenate(self.mats, axis=1)


def _prep_consts(hl0_W, hl0_b, hl_W, hl_b, g0_W, g0_b, gp_W, gp_b,
                 gf_W, gf_b, fc0_W, fc0_b, fc1_W, fc1_b):
    f64 = np.float64
    hl0_W, hl_W, g0_W, gp_W, gf_W, fc0_W, fc1_W = [
        np.asarray(a, f64) for a in (hl0_W, hl_W, g0_W, gp_W, gf_W, fc0_W, fc1_W)]

    A06 = np.zeros((4, 6), f64)   # layer0 agg: 6 nodes -> 4 distinct
    A44 = np.zeros((4, 4), f64)   # layers1-3 agg: 4 -> 4 with multiplicity
    for i, di in enumerate(DNODES):
        for j in range(6):
            A06[i, j] = ADJ[di, j] / DEG[di]
        for j, dj in enumerate(DNODES):
            A44[i, j] = ADJ[di, dj] * MULT[j] / DEG[di]

    wp = _Pack(ACT_NP)
    wp.add("ident", np.eye(128))

    # L0 proj0 (3->8 per node), halves A (groups 0,1) and B (groups 2,3)
    for half, gs in (("A", (0, 1)), ("B", (2, 3))):
        w = np.zeros((128, 112), f64)
        for gi, g in enumerate(gs):
            for n in range(6):
                for f in range(3):
                    for h in range(8):
                        w[32 * g + 3 * n + f, 64 * gi + 8 * n + h] = g0_W[f, h]
        wp.add(f"l0p0{half}", w)

    # L0 proj1/proj2 (8->8 per node), same matrix for both halves
    for pi, k in (("l0p1", 0), ("l0p2", 1)):
        w = np.zeros((112, 112), f64)
        for gi in range(2):
            for n in range(6):
                w[64 * gi + 8 * n:64 * gi + 8 * n + 8,
                  64 * gi + 8 * n:64 * gi + 8 * n + 8] = gp_W[k]
        wp.add(pi, w)

    # L0 agg: [6 nodes x 8] -> [4 distinct x 8], per half (out base differs)
    w = np.zeros((112, 64), f64)
    for gi in range(2):
        for ip in range(4):
            for j in range(6):
                for f in range(8):
                    w[64 * gi + 8 * j + f, 32 * gi + 8 * ip + f] = A06[ip, j]
    wp.add("l0agg", w)

    # L1..3 proj + agg, 4-group block diagonal [128,128]
    for l in range(1, 4):
        for p in range(3):
            k = 2 + 3 * (l - 1) + p
            w = np.zeros((128, 128), f64)
            for g in range(4):
                for n in range(4):
                    w[32 * g + 8 * n:32 * g + 8 * n + 8,
                      32 * g + 8 * n:32 * g + 8 * n + 8] = gp_W[k]
            wp.add(f"l{l}p{p}", w)
        w = np.zeros((128, 128), f64)
        for g in range(4):
            for ip in range(4):
                for j in range(4):
                    for f in range(8):
                        w[32 * g + 8 * j + f, 32 * g + 8 * ip + f] = A44[ip, j]
        wp.add(f"l{l}agg", w)

    # aux branch
    w = np.zeros((128, 32), f64)
    for g in range(4):
        for q in range(5):
            for h in range(8):
                w[32 * g + 18 + q, 8 * g + h] = hl0_W[q, h]
    wp.add("hl0", w)
    for i in range(3):
        w = np.zeros((32, 32), f64)
        for g in range(4):
            w[8 * g:8 * g + 8, 8 * g:8 * g + 8] = hl_W[i]
        wp.add(f"hl{i + 1}", w)

    # gf (48->8) folded over duplicate nodes
    gfW4 = [gf_W[0:8], gf_W[8:16], gf_W[16:24] + gf_W[24:32],
            gf_W[32:40] + gf_W[40:48]]
    w = np.zeros((128, 32), f64)
    for g in range(4):
        for n in range(4):
            for f in range(8):
                for h in range(8):
                    w[32 * g + 8 * n + f, 8 * g + h] = gfW4[n][f, h]
    wp.add("gf", w)

    # fc0: rows 0..31 = h (fc0_W rows 8..15), rows 32..63 = g (rows 0..7)
    w = np.zeros((64, 64), f64)
    for g in range(4):
        for j in range(8):
            for m in range(16):
                w[8 * g + j, 16 * g + m] = fc0_W[8 + j, m]
                w[32 + 8 * g + j, 16 * g + m] = fc0_W[j, m]
    wp.add("fc0", w)

    # fc1 folded with the log_softmax difference matrix
    dw = fc1_W[:, 1] - fc1_W[:, 0]
    w = np.zeros((64, 8), f64)
    for g in range(4):
        for m in range(16):
            w[16 * g + m, 2 * g] = dw[m]
            w[16 * g + m, 2 * g + 1] = -dw[m]
    wp.add("fc1d", w)

    # biases
    bp = _Pack()
    negI = np.zeros((128, 8), f64)
    for m in range(4):
        for k in range(8):
            negI[32 * m + k, k] = -1.0
    bp.add("negI", negI)
    b = np.zeros((112, 1), f64)
    for gi in range(2):
        for n in range(6):
            b[64 * gi + 8 * n:64 * gi + 8 * n + 8, 0] = g0_b
    bp.add("l0p0", b)
    for nm, k in (("l0p1", 0), ("l0p2", 1)):
        b = np.zeros((112, 1), f64)
        for gi in range(2):
            for n in range(6):
                b[64 * gi + 8 * n:64 * gi + 8 * n + 8, 0] = gp_b[k]
        bp.add(nm, b)
    for l in range(1, 4):
        for p in range(3):
            k = 2 + 3 * (l - 1) + p
            b = np.zeros((128, 1), f64)
            for g in range(4):
                for n in range(4):
                    b[32 * g + 8 * n:32 * g + 8 * n + 8, 0] = gp_b[k]
            bp.add(f"l{l}p{p}", b)
    b = np.zeros((32, 1), f64)
    for g in range(4):
        b[8 * g:8 * g + 8, 0] = hl0_b
    bp.add("hl0", b)
    for i in range(2):
        b = np.zeros((32, 1), f64)
        for g in range(4):
            b[8 * g:8 * g + 8, 0] = hl_b[i]
        bp.add(f"hl{i + 1}", b)
    b = np.zeros((64, 1), f64)
    for g in range(4):
        b[8 * g:8 * g + 8, 0] = hl_b[2]
        b[32 + 8 * g:32 + 8 * g + 8, 0] = gf_b
    bp.add("Z", b)
    b = np.zeros((64, 1), f64)
    for g in range(4):
        b[16 * g:16 * g + 16, 0] = fc0_b
    bp.add("fc0", b)
    delta = float(fc1_b[1]) - float(fc1_b[0])
    b = np.zeros((104, 1), f64)
    for m in range(4):
        for g in range(4):
            b[32 * m + 2 * g, 0] = delta
            b[32 * m + 2 * g + 1, 0] = -delta
    bp.add("head", b)

    return wp, bp


# --------------------------------------------------------------------------
# Device program
# --------------------------------------------------------------------------

def _build_program(wslices, bslices, wcols, bcols, nst=NST):
    nc = bacc.Bacc("TRN2", target_bir_lowering=False, debug=False,
                   enable_asserts=False, num_devices=NCORES)
    bc = nst * ST_EV
    x_d = nc.dram_tensor("x", [bc, 18], F32, kind="ExternalInput")
    o_d = nc.dram_tensor("o", [bc, 5], F32, kind="ExternalInput")
    w_d = nc.dram_tensor("wpack", [128, wcols], ACT_DT, kind="ExternalInput")
    b_d = nc.dram_tensor("bpack", [128, bcols], F32, kind="ExternalInput")
    y_d = nc.dram_tensor("y", [bc, 2], F32, kind="ExternalOutput")

    xv = x_d.ap().rearrange("(s p e) f -> s p (e f)", s=nst, p=128, e=64)
    ov = o_d.ap().rearrange("(s p e) f -> s p (e f)", s=nst, p=128, e=64)
    yv = y_d.ap().rearrange("(s p e) j -> s p (e j)", s=nst, p=128, e=64)

    with TileCtx(nc) as (tc, ctx):
        wpool = ctx.enter_context(tc.tile_pool(name="wts", bufs=1))
        ldpool = ctx.enter_context(tc.tile_pool(name="loads", bufs=2))
        stgpool = ctx.enter_context(tc.tile_pool(name="stg", bufs=2))
        acts = ctx.enter_context(tc.tile_pool(name="acts", bufs=10))
        opool = ctx.enter_context(tc.tile_pool(name="outs", bufs=2))
        psum = ctx.enter_context(
            tc.tile_pool(name="ps", bufs=5, space="PSUM"))
        dpool = ctx.enter_context(
            tc.tile_pool(name="dps", bufs=3, space="PSUM"))

        wt = wpool.tile([128, wcols], ACT_DT)
        nc.sync.dma_start(out=wt[:], in_=w_d.ap())
        bt = wpool.tile([128, bcols], F32)
        nc.sync.dma_start(out=bt[:], in_=b_d.ap())

        def W(name, rows):
            c0, cw = wslices[name]
            return wt[0:rows, c0:c0 + cw]

        def WB(name, r0, rows):
            c0, cw = bslices[name]
            return bt[r0:r0 + rows, c0:c0 + cw]

        def Bias(name, rows):
            c0, cw = bslices[name]
            return bt[0:rows, c0:c0 + 1]

        def prelu(dst, src, bias):
            nc.scalar.activation(dst, src, AF.Prelu, bias=bias, scale=1.0,
                                 alpha=ALPHA)

        for st in range(nst):
            xl = ldpool.tile([128, 64 * 18], F32, tag="xl")
            nc.sync.dma_start(out=xl[:], in_=xv[st])
            olt = ldpool.tile([128, 64 * 5], F32, tag="ol")
            nc.sync.dma_start(out=olt[:], in_=ov[st])

            # repack into 32-col event slots (cols 23..31 zeroed)
            stg = stgpool.tile([128, 64 * 32], ACT_DT, tag="stg")
            s3 = stg[:].rearrange("p (e f) -> p e f", f=32)
            nc.vector.memset(s3[:, :, 23:32], 0.0)
            nc.vector.tensor_copy(
                s3[:, :, 0:18],
                xl[:].rearrange("p (e f) -> p e f", f=18))
            nc.vector.tensor_copy(
                s3[:, :, 18:23],
                olt[:].rearrange("p (e f) -> p e f", f=5))

            D_ps = dpool.tile([128, NCOL], F32, tag="dps")
            for m in range(NPASS):
                # ---- transpose 16 chunks into feature-major psum ----
                T_in = psum.tile([128, NCOL], F32, tag="ps")
                for c in range(4):
                    for g in range(4):
                        eps = 16 * m + 4 * c + g
                        nc.tensor.matmul(
                            T_in[32 * g:32 * g + 32, 128 * c:128 * c + 128],
                            stg[:, 32 * eps:32 * eps + 32],
                            wt[0:128, 0:128],  # identity
                            start=True, stop=True,
                            tile_position=(0, 32 * g))
                in0 = acts.tile([128, NCOL], ACT_DT, tag="act")
                nc.vector.tensor_copy(in0[:], T_in[:])

                # ---- GNN layer 0 (6 nodes, two halves of 2 groups) ----
                shalves = []
                for half, wname in ((0, "l0p0A"), (1, "l0p0B")):
                    p0 = psum.tile([112, NCOL], F32, tag="ps")
                    nc.tensor.matmul(p0[:], W(wname, 119), in0[0:119, :],
                                     start=True, stop=True)
                    s0 = acts.tile([112, NCOL], ACT_DT, tag="act")
                    prelu(s0[:], p0[:], Bias("l0p0", 112))
                    p1 = psum.tile([112, NCOL], F32, tag="ps")
                    nc.tensor.matmul(p1[:], W("l0p1", 112), s0[:],
                                     start=True, stop=True)
                    s1 = acts.tile([112, NCOL], ACT_DT, tag="act")
                    prelu(s1[:], p1[:], Bias("l0p1", 112))
                    p2 = psum.tile([112, NCOL], F32, tag="ps")
                    nc.tensor.matmul(p2[:], W("l0p2", 112), s1[:],
                                     start=True, stop=True)
                    s2 = acts.tile([112, NCOL], ACT_DT, tag="act")
                    prelu(s2[:], p2[:], Bias("l0p2", 112))
                    shalves.append(s2)
                pagg = psum.tile([128, NCOL], F32, tag="ps")
                nc.tensor.matmul(pagg[0:64, :], W("l0agg", 112),
                                 shalves[0][:], start=True, stop=True)
                nc.tensor.matmul(pagg[64:128, :], W("l0agg", 112),
                                 shalves[1][:], start=True, stop=True)
                G = acts.tile([128, NCOL], ACT_DT, tag="act")
                prelu(G[:], pagg[:], 0.0)

                # ---- GNN layers 1..3 (4 distinct nodes, 4 groups) ----
                for l in range(1, 4):
                    for p in range(3):
                        pp = psum.tile([128, NCOL], F32, tag="ps")
                        nc.tensor.matmul(pp[:], W(f"l{l}p{p}", 128), G[:],
                                         start=True, stop=True)
                        Gn = acts.tile([128, NCOL], ACT_DT, tag="act")
                        prelu(Gn[:], pp[:], Bias(f"l{l}p{p}", 128))
                        G = Gn
                    pa = psum.tile([128, NCOL], F32, tag="ps")
                    nc.tensor.matmul(pa[:], W(f"l{l}agg", 128), G[:],
                                     start=True, stop=True)
                    G = acts.tile([128, NCOL], ACT_DT, tag="act")
                    prelu(G[:], pa[:], 0.0)

                # ---- aux branch ----
                ph = psum.tile([32, NCOL], F32, tag="ps")
                nc.tensor.matmul(ph[:], W("hl0", 119), in0[0:119, :],
                                 start=True, stop=True)
                h = acts.tile([32, NCOL], ACT_DT, tag="act")
                prelu(h[:], ph[:], Bias("hl0", 32))
                for i in (1, 2):
                    ph = psum.tile([32, NCOL], F32, tag="ps")
                    nc.tensor.matmul(ph[:], W(f"hl{i}", 32), h[:],
                                     start=True, stop=True)
                    hn = acts.tile([32, NCOL], ACT_DT, tag="act")
                    prelu(hn[:], ph[:], Bias(f"hl{i}", 32))
                    h = hn

                # ---- heads: Z = [hl3 | gf], fc0, fc1d ----
                Z = psum.tile([64, NCOL], F32, tag="ps")
                nc.tensor.matmul(Z[0:32, :], W("hl3", 32), h[:],
                                 start=True, stop=True)
                nc.tensor.matmul(Z[32:64, :], W("gf", 128), G[:],
                                 start=True, stop=True)
                zin = acts.tile([64, NCOL], ACT_DT, tag="act")
                prelu(zin[:], Z[:], Bias("Z", 64))
                pz = psum.tile([64, NCOL], F32, tag="ps")
                nc.tensor.matmul(pz[:], W("fc0", 64), zin[:],
                                 start=True, stop=True)
                z2 = acts.tile([64, NCOL], ACT_DT, tag="act")
                prelu(z2[:], pz[:], Bias("fc0", 64))
                nc.tensor.matmul(D_ps[32 * m:32 * m + 8, :],
                                 W("fc1d", 64), z2[:], start=True, stop=True,
                                 tile_position=(0, 32 * m))

            # ---- per-supertile head: -softplus, transpose out, store ----
            bh = Bias("head", 104)
            ab = acts.tile([104, NCOL], F32, tag="act")
            nc.scalar.activation(ab[:], D_ps[0:104, :], AF.Abs, bias=bh)
            ex = acts.tile([104, NCOL], F32, tag="act")
            nc.scalar.activation(ex[:], ab[:], AF.Exp, scale=-1.0)
            lnt = acts.tile([104, NCOL], F32, tag="act")
            nc.scalar.activation(lnt[:], ex[:], AF.Ln, bias=1.0)
            ret = acts.tile([104, NCOL], F32, tag="act")
            nc.scalar.activation(ret[:], D_ps[0:104, :], AF.Relu, bias=bh)
            sm = acts.tile([104, NCOL], F32, tag="act")
            nc.vector.tensor_add(sm[:], lnt[:], ret[:])

            OutPs = dpool.tile([128, 128], F32, tag="dps")
            for m in range(NPASS):
                for c in range(4):
                    nc.tensor.matmul(
                        OutPs[:, 32 * m + 8 * c:32 * m + 8 * c + 8],
                        sm[32 * m:32 * m + 8, 128 * c:128 * c + 128],
                        WB("negI", 32 * m, 8),
                        start=True, stop=True,
                        tile_position=(32 * m, 0))
            outsb = opool.tile([128, 128], F32, tag="osb")
            nc.vector.tensor_copy(outsb[:], OutPs[:])
            nc.sync.dma_start(out=yv[st], in_=outsb[:])

    nc.compile()
    return nc


class TileCtx:
    """TileContext + ExitStack in one `with`."""

    def __init__(self, nc):
        self.nc = nc

    def __enter__(self):
        self.ctx = ExitStack()
        self.tc = tile.TileContext(self.nc)
        self.tc.__enter__()
        return self.tc, self.ctx

    def __exit__(self, *a):
        self.ctx.close()
        return self.tc.__exit__(*a)


_CACHE = {}


def _get_program(wslices_key, wcols, bcols, nst, wslices, bslices):
    key = (wcols, bcols, nst)
    if key not in _CACHE:
        _CACHE[key] = _build_program(wslices, bslices, wcols, bcols, nst)
    return _CACHE[key]


def kernel(x, o, hl0_W, hl0_b, hl_W, hl_b, g0_W, g0_b, gp_W, gp_b,
           gf_W, gf_b, fc0_W, fc0_b, fc1_W, fc1_b, _nst=NST, _trace=False,
           _tmpdir=None):
    wp, bp = _prep_consts(hl0_W, hl0_b, hl_W, hl_b, g0_W, g0_b, gp_W, gp_b,
                          gf_W, gf_b, fc0_W, fc0_b, fc1_W, fc1_b)
    wpack = wp.array()
    assert wpack.dtype == ACT_NP
    bpack = bp.array()
    nc = _get_program(None, wpack.shape[1], bpack.shape[1], _nst,
                      wp.slices, bp.slices)

    bc = _nst * ST_EV
    ncores_used = min(NCORES, (np.asarray(x).shape[0] + bc - 1) // bc)
    x = np.ascontiguousarray(np.asarray(x, np.float32)).reshape(-1, 18)
    o = np.ascontiguousarray(np.asarray(o, np.float32)).reshape(-1, 5)

    in_maps = []
    for c in range(ncores_used):
        in_maps.append({
            "x": x[c * bc:(c + 1) * bc],
            "o": o[c * bc:(c + 1) * bc],
            "wpack": wpack,
            "bpack": bpack,
        })
    res = run_bass_kernel_spmd(nc, in_maps, list(range(ncores_used)),
                               trace=_trace, tmpdir=_tmpdir)
    ys = [res.results[c]["y"] for c in range(ncores_used)]
    out = np.concatenate(ys, axis=0)
    if _trace:
        kernel._last_exec_ns = res.exec_time_ns
    return out


kernel._last_exec_ns = None


# revision 9
# speedup vs baseline: 3.5586x; 1.7983x over previous
"""Trainium2 Bass kernel for nn_BBTT_GNN (gnn_message_passing).

Strategy (pure data parallel, 8 cores, batch-sharded):
  - Feature-major on-chip layout: features on SBUF partitions, events on the
    free dim.  4 event "groups" live in the 4 32-partition blocks, so every
    matmul processes 4*512 = 2048 events per N=512 instruction.
  - All per-node 8x8 layers are packed into block-diagonal 128x128 lhsT
    weights (host-prepped); aggregation (adj@n/deg) is folded into a matmul.
  - Graph dedup: ADJ rows/cols 2==3 and 4==5, so after the first aggregation
    only 4 distinct node states exist -> layers 1..3 use 32 feats/group.
  - leaky_relu(0.01) + bias fused into one ScalarE Prelu pass per layer.
  - log_softmax(2) = -(relu(d)+log1p(exp(-|d|))) with d = +/-(v1-v0), done
    with Abs/Exp/Ln/Relu ACT passes once per supertile.
  - PE transposes (regular matmuls vs identity) move events<->features at
    entry/exit so all DMAs are large and contiguous.

Event mapping inside a supertile of 8192 events:
  event = st*8192 + 64*p + eps,  eps = 16*m + 4*c + g
  (p: transpose column 0..127, m: MM-pass 0..3, c: 128-col chunk 0..3,
   g: partition group 0..3).  This makes every DMA (x, o loads and y store)
  contiguous per partition.
"""

import os
import sys
from contextlib import ExitStack

for _p in ("/opt/trn_rl_repo", "/root/.axon_site/_ro/trn_rl_repo"):
    if os.path.isdir(_p) and _p not in sys.path:
        sys.path.insert(0, _p)

import numpy as np

import concourse.bass as bass
import concourse.tile as tile
from concourse import bacc, mybir
from concourse.bass_utils import run_bass_kernel_spmd

F32 = mybir.dt.float32
F16 = mybir.dt.float16
AF = mybir.ActivationFunctionType
ACT_DT = F16          # dtype for matmul operands (weights/activations)
ACT_NP = np.float16

NCORES = 8
B_TOTAL = 1048576
BC = B_TOTAL // NCORES          # 131072 events per core
ST_EV = 8192                    # events per supertile
NST = BC // ST_EV               # 16 supertiles
NPASS = 4                       # MM passes per supertile (2048 events each)
NCOL = 512                      # matmul free dim

ADJ = np.array(
    [[1, 1, 1, 1, 0, 0], [1, 1, 0, 0, 1, 1], [1, 0, 1, 1, 0, 0],
     [1, 0, 1, 1, 0, 0], [0, 1, 0, 0, 1, 1], [0, 1, 0, 0, 1, 1]],
    dtype=np.float64)
DEG = ADJ.sum(axis=1)           # [4,4,3,3,3,3]
DNODES = [0, 1, 2, 4]           # distinct nodes after first aggregation
MULT = [1, 1, 2, 2]
ALPHA = 0.01


# --------------------------------------------------------------------------
# Host-side constant packing
# --------------------------------------------------------------------------

class _Pack:
    def __init__(self, np_dtype=np.float32):
        self.np_dtype = np_dtype
        self.cols = 0
        self.slices = {}
        self.mats = []

    def add(self, name, mat):
        mat = np.asarray(mat, dtype=self.np_dtype)
        assert mat.shape[0] <= 128
        m = np.zeros((128, mat.shape[1]), self.np_dtype)
        m[: mat.shape[0]] = mat
        self.slices[name] = (self.cols, mat.shape[1])
        self.mats.append(m)
        self.cols += mat.shape[1]

    def array(self):
        return np.concatenate(self.mats, axis=1)


def _prep_consts(hl0_W, hl0_b, hl_W, hl_b, g0_W, g0_b, gp_W, gp_b,
                 gf_W, gf_b, fc0_W, fc0_b, fc1_W, fc1_b):
    f64 = np.float64
    hl0_W, hl_W, g0_W, gp_W, gf_W, fc0_W, fc1_W = [
        np.asarray(a, f64) for a in (hl0_W, hl_W, g0_W, gp_W, gf_W, fc0_W, fc1_W)]

    A06 = np.zeros((4, 6), f64)   # layer0 agg: 6 nodes -> 4 distinct
    A44 = np.zeros((4, 4), f64)   # layers1-3 agg: 4 -> 4 with multiplicity
    for i, di in enumerate(DNODES):
        for j in range(6):
            A06[i, j] = ADJ[di, j] / DEG[di]
        for j, dj in enumerate(DNODES):
            A44[i, j] = ADJ[di, dj] * MULT[j] / DEG[di]

    wp = _Pack(ACT_NP)
    wp.add("ident", np.eye(128))

    # L0 proj0 (3->8 per node), halves A (groups 0,1) and B (groups 2,3)
    for half, gs in (("A", (0, 1)), ("B", (2, 3))):
        w = np.zeros((128, 112), f64)
        for gi, g in enumerate(gs):
            for n in range(6):
                for f in range(3):
                    for h in range(8):
                        w[32 * g + 3 * n + f, 64 * gi + 8 * n + h] = g0_W[f, h]
        wp.add(f"l0p0{half}", w)

    # L0 proj1/proj2 (8->8 per node), same matrix for both halves
    for pi, k in (("l0p1", 0), ("l0p2", 1)):
        w = np.zeros((112, 112), f64)
        for gi in range(2):
            for n in range(6):
                w[64 * gi + 8 * n:64 * gi + 8 * n + 8,
                  64 * gi + 8 * n:64 * gi + 8 * n + 8] = gp_W[k]
        wp.add(pi, w)

    # L0 agg: [6 nodes x 8] -> [4 distinct x 8], per half (out base differs)
    w = np.zeros((112, 64), f64)
    for gi in range(2):
        for ip in range(4):
            for j in range(6):
                for f in range(8):
                    w[64 * gi + 8 * j + f, 32 * gi + 8 * ip + f] = A06[ip, j]
    wp.add("l0agg", w)

    # L1..3 proj + agg, 4-group block diagonal [128,128]
    for l in range(1, 4):
        for p in range(3):
            k = 2 + 3 * (l - 1) + p
            w = np.zeros((128, 128), f64)
            for g in range(4):
                for n in range(4):
                    w[32 * g + 8 * n:32 * g + 8 * n + 8,
                      32 * g + 8 * n:32 * g + 8 * n + 8] = gp_W[k]
            wp.add(f"l{l}p{p}", w)
        w = np.zeros((128, 128), f64)
        for g in range(4):
            for ip in range(4):
                for j in range(4):
                    for f in range(8):
                        w[32 * g + 8 * j + f, 32 * g + 8 * ip + f] = A44[ip, j]
        wp.add(f"l{l}agg", w)

    # aux branch
    w = np.zeros((128, 32), f64)
    for g in range(4):
        for q in range(5):
            for h in range(8):
                w[32 * g + 18 + q, 8 * g + h] = hl0_W[q, h]
    wp.add("hl0", w)
    for i in range(3):
        w = np.zeros((32, 32), f64)
        for g in range(4):
            w[8 * g:8 * g + 8, 8 * g:8 * g + 8] = hl_W[i]
        wp.add(f"hl{i + 1}", w)

    # gf (48->8) folded over duplicate nodes
    gfW4 = [gf_W[0:8], gf_W[8:16], gf_W[16:24] + gf_W[24:32],
            gf_W[32:40] + gf_W[40:48]]
    w = np.zeros((128, 32), f64)
    for g in range(4):
        for n in range(4):
            for f in range(8):
                for h in range(8):
                    w[32 * g + 8 * n + f, 8 * g + h] = gfW4[n][f, h]
    wp.add("gf", w)

    # fc0: rows 0..31 = h (fc0_W rows 8..15), rows 32..63 = g (rows 0..7)
    w = np.zeros((64, 64), f64)
    for g in range(4):
        for j in range(8):
            for m in range(16):
                w[8 * g + j, 16 * g + m] = fc0_W[8 + j, m]
                w[32 + 8 * g + j, 16 * g + m] = fc0_W[j, m]
    wp.add("fc0", w)

    # fc1 folded with the log_softmax difference matrix
    dw = fc1_W[:, 1] - fc1_W[:, 0]
    w = np.zeros((64, 8), f64)
    for g in range(4):
        for m in range(16):
            w[16 * g + m, 2 * g] = dw[m]
            w[16 * g + m, 2 * g + 1] = -dw[m]
    wp.add("fc1d", w)

    # biases
    bp = _Pack()
    negI = np.zeros((128, 8), f64)
    for m in range(4):
        for k in range(8):
            negI[32 * m + k, k] = -1.0
    bp.add("negI", negI)
    b = np.zeros((112, 1), f64)
    for gi in range(2):
        for n in range(6):
            b[64 * gi + 8 * n:64 * gi + 8 * n + 8, 0] = g0_b
    bp.add("l0p0", b)
    for nm, k in (("l0p1", 0), ("l0p2", 1)):
        b = np.zeros((112, 1), f64)
        for gi in range(2):
            for n in range(6):
                b[64 * gi + 8 * n:64 * gi + 8 * n + 8, 0] = gp_b[k]
        bp.add(nm, b)
    for l in range(1, 4):
        for p in range(3):
            k = 2 + 3 * (l - 1) + p
            b = np.zeros((128, 1), f64)
            for g in range(4):
                for n in range(4):
                    b[32 * g + 8 * n:32 * g + 8 * n + 8, 0] = gp_b[k]
            bp.add(f"l{l}p{p}", b)
    b = np.zeros((32, 1), f64)
    for g in range(4):
        b[8 * g:8 * g + 8, 0] = hl0_b
    bp.add("hl0", b)
    for i in range(2):
        b = np.zeros((32, 1), f64)
        for g in range(4):
            b[8 * g:8 * g + 8, 0] = hl_b[i]
        bp.add(f"hl{i + 1}", b)
    b = np.zeros((64, 1), f64)
    for g in range(4):
        b[8 * g:8 * g + 8, 0] = hl_b[2]
        b[32 + 8 * g:32 + 8 * g + 8, 0] = gf_b
    bp.add("Z", b)
    b = np.zeros((64, 1), f64)
    for g in range(4):
        b[16 * g:16 * g + 16, 0] = fc0_b
    bp.add("fc0", b)
    delta = float(fc1_b[1]) - float(fc1_b[0])
    b = np.zeros((104, 1), f64)
    for m in range(4):
        for g in range(4):
            b[32 * m + 2 * g, 0] = delta
            b[32 * m + 2 * g + 1, 0] = -delta
    bp.add("head", b)

    return wp, bp


# --------------------------------------------------------------------------
# Device program
# --------------------------------------------------------------------------

def _build_program(wslices, bslices, wcols, bcols, nst=NST):
    nc = bacc.Bacc("TRN2", target_bir_lowering=False, debug=False,
                   enable_asserts=False, num_devices=NCORES)
    bc = nst * ST_EV
    x_d = nc.dram_tensor("x", [bc, 18], F32, kind="ExternalInput")
    o_d = nc.dram_tensor("o", [bc, 5], F32, kind="ExternalInput")
    w_d = nc.dram_tensor("wpack", [128, wcols], ACT_DT, kind="ExternalInput")
    b_d = nc.dram_tensor("bpack", [128, bcols], F32, kind="ExternalInput")
    y_d = nc.dram_tensor("y", [bc, 2], F32, kind="ExternalOutput")

    xv = x_d.ap().rearrange("(s p e) f -> s p (e f)", s=nst, p=128, e=64)
    ov = o_d.ap().rearrange("(s p e) f -> s p (e f)", s=nst, p=128, e=64)
    yv = y_d.ap().rearrange("(s p e) j -> s p (e j)", s=nst, p=128, e=64)

    with TileCtx(nc) as (tc, ctx):
        wpool = ctx.enter_context(tc.tile_pool(name="wts", bufs=1))
        ldpool = ctx.enter_context(tc.tile_pool(name="loads", bufs=2))
        stgpool = ctx.enter_context(tc.tile_pool(name="stg", bufs=2))
        acts = ctx.enter_context(tc.tile_pool(name="acts", bufs=16))
        opool = ctx.enter_context(tc.tile_pool(name="outs", bufs=2))
        psum = ctx.enter_context(
            tc.tile_pool(name="ps", bufs=5, space="PSUM"))
        dpool = ctx.enter_context(
            tc.tile_pool(name="dps", bufs=3, space="PSUM"))

        wt = wpool.tile([128, wcols], ACT_DT)
        nc.sync.dma_start(out=wt[:], in_=w_d.ap())
        bt = wpool.tile([128, bcols], F32)
        nc.sync.dma_start(out=bt[:], in_=b_d.ap())

        def W(name, rows):
            c0, cw = wslices[name]
            return wt[0:rows, c0:c0 + cw]

        def WB(name, r0, rows):
            c0, cw = bslices[name]
            return bt[r0:r0 + rows, c0:c0 + cw]

        def Bias(name, rows):
            c0, cw = bslices[name]
            return bt[0:rows, c0:c0 + 1]

        def prelu(dst, src, bias):
            nc.scalar.activation(dst, src, AF.Prelu, bias=bias, scale=1.0,
                                 alpha=ALPHA)

        def pass_gen(stg, D_ps, m):
            """One MM-pass (512 cols x 4 groups = 2048 events); yields after
            each dependent unit so two passes can interleave engine streams."""
            T_in = psum.tile([128, NCOL], F32, tag="ps")
            for c in range(4):
                for g in range(4):
                    eps = 16 * m + 4 * c + g
                    nc.tensor.matmul(
                        T_in[32 * g:32 * g + 32, 128 * c:128 * c + 128],
                        stg[:, 32 * eps:32 * eps + 32],
                        wt[0:128, 0:128],  # identity
                        start=True, stop=True,
                        tile_position=(0, 32 * g))
            yield
            in0 = acts.tile([128, NCOL], ACT_DT, tag="act")
            nc.vector.tensor_copy(in0[:], T_in[:])
            yield

            # GNN layer 0 (6 nodes, two halves of 2 groups each)
            shalves = []
            for half, wname in ((0, "l0p0A"), (1, "l0p0B")):
                p0 = psum.tile([112, NCOL], F32, tag="ps")
                nc.tensor.matmul(p0[:], W(wname, 119), in0[0:119, :],
                                 start=True, stop=True)
                s0 = acts.tile([112, NCOL], ACT_DT, tag="act")
                prelu(s0[:], p0[:], Bias("l0p0", 112))
                yield
                p1 = psum.tile([112, NCOL], F32, tag="ps")
                nc.tensor.matmul(p1[:], W("l0p1", 112), s0[:],
                                 start=True, stop=True)
                s1 = acts.tile([112, NCOL], ACT_DT, tag="act")
                prelu(s1[:], p1[:], Bias("l0p1", 112))
                yield
                p2 = psum.tile([112, NCOL], F32, tag="ps")
                nc.tensor.matmul(p2[:], W("l0p2", 112), s1[:],
                                 start=True, stop=True)
                s2 = acts.tile([112, NCOL], ACT_DT, tag="act")
                prelu(s2[:], p2[:], Bias("l0p2", 112))
                yield
                shalves.append(s2)
            pagg = psum.tile([128, NCOL], F32, tag="ps")
            nc.tensor.matmul(pagg[0:64, :], W("l0agg", 112),
                             shalves[0][:], start=True, stop=True)
            nc.tensor.matmul(pagg[64:128, :], W("l0agg", 112),
                             shalves[1][:], start=True, stop=True)
            G = acts.tile([128, NCOL], ACT_DT, tag="act")
            prelu(G[:], pagg[:], 0.0)
            yield

            # aux branch (interleaved between GNN layers for engine overlap)
            ph = psum.tile([32, NCOL], F32, tag="ps")
            nc.tensor.matmul(ph[:], W("hl0", 119), in0[0:119, :],
                             start=True, stop=True)
            h = acts.tile([32, NCOL], ACT_DT, tag="act")
            prelu(h[:], ph[:], Bias("hl0", 32))
            yield

            # GNN layers 1..3 (4 distinct nodes, 4 groups)
            for l in range(1, 4):
                for p in range(3):
                    pp = psum.tile([128, NCOL], F32, tag="ps")
                    nc.tensor.matmul(pp[:], W(f"l{l}p{p}", 128), G[:],
                                     start=True, stop=True)
                    Gn = acts.tile([128, NCOL], ACT_DT, tag="act")
                    prelu(Gn[:], pp[:], Bias(f"l{l}p{p}", 128))
                    G = Gn
                    yield
                pa = psum.tile([128, NCOL], F32, tag="ps")
                nc.tensor.matmul(pa[:], W(f"l{l}agg", 128), G[:],
                                 start=True, stop=True)
                G = acts.tile([128, NCOL], ACT_DT, tag="act")
                prelu(G[:], pa[:], 0.0)
                yield
                if l == 1:
                    ph = psum.tile([32, NCOL], F32, tag="ps")
                    nc.tensor.matmul(ph[:], W("hl1", 32), h[:],
                                     start=True, stop=True)
                    hn = acts.tile([32, NCOL], ACT_DT, tag="act")
                    prelu(hn[:], ph[:], Bias("hl1", 32))
                    h = hn
                    yield
                if l == 2:
                    ph = psum.tile([32, NCOL], F32, tag="ps")
                    nc.tensor.matmul(ph[:], W("hl2", 32), h[:],
                                     start=True, stop=True)
                    hn = acts.tile([32, NCOL], ACT_DT, tag="act")
                    prelu(hn[:], ph[:], Bias("hl2", 32))
                    h = hn
                    yield

            # heads: Z = [hl3 | gf], fc0, fc1d
            Z = psum.tile([64, NCOL], F32, tag="ps")
            nc.tensor.matmul(Z[0:32, :], W("hl3", 32), h[:],
                             start=True, stop=True)
            nc.tensor.matmul(Z[32:64, :], W("gf", 128), G[:],
                             start=True, stop=True)
            zin = acts.tile([64, NCOL], ACT_DT, tag="act")
            prelu(zin[:], Z[:], Bias("Z", 64))
            yield
            pz = psum.tile([64, NCOL], F32, tag="ps")
            nc.tensor.matmul(pz[:], W("fc0", 64), zin[:],
                             start=True, stop=True)
            z2 = acts.tile([64, NCOL], ACT_DT, tag="act")
            prelu(z2[:], pz[:], Bias("fc0", 64))
            yield
            nc.tensor.matmul(D_ps[32 * m:32 * m + 8, :],
                             W("fc1d", 64), z2[:], start=True, stop=True,
                             tile_position=(0, 32 * m))
            yield

        def head_gen(D_ps, st):
            """-softplus + output transpose + store for one supertile."""
            bh = Bias("head", 104)
            ab = acts.tile([104, NCOL], F32, tag="hact")
            nc.scalar.activation(ab[:], D_ps[0:104, :], AF.Abs, bias=bh)
            yield
            ret = acts.tile([104, NCOL], F32, tag="hact")
            nc.scalar.activation(ret[:], D_ps[0:104, :], AF.Relu, bias=bh)
            yield
            ex = acts.tile([104, NCOL], F32, tag="hact")
            nc.scalar.activation(ex[:], ab[:], AF.Exp, scale=-1.0)
            yield
            lnt = acts.tile([104, NCOL], F32, tag="hact")
            nc.scalar.activation(lnt[:], ex[:], AF.Ln, bias=1.0)
            yield
            sm = acts.tile([104, NCOL], F32, tag="hact")
            nc.vector.tensor_add(sm[:], lnt[:], ret[:])
            yield
            OutPs = dpool.tile([128, 128], F32, tag="dps")
            for m in range(NPASS):
                for c in range(4):
                    nc.tensor.matmul(
                        OutPs[:, 32 * m + 8 * c:32 * m + 8 * c + 8],
                        sm[32 * m:32 * m + 8, 128 * c:128 * c + 128],
                        WB("negI", 32 * m, 8),
                        start=True, stop=True,
                        tile_position=(32 * m, 0))
                yield
            outsb = opool.tile([128, 128], F32, tag="osb")
            nc.vector.tensor_copy(outsb[:], OutPs[:])
            nc.sync.dma_start(out=yv[st], in_=outsb[:])

        def drive(gens):
            gens = [g for g in gens if g is not None]
            while gens:
                nxt = []
                for g in gens:
                    try:
                        next(g)
                        nxt.append(g)
                    except StopIteration:
                        continue
                    except TypeError:
                        continue
                gens = nxt

        carry = None
        for st in range(nst):
            xl = ldpool.tile([128, 64 * 18], F32, tag="xl")
            nc.sync.dma_start(out=xl[:], in_=xv[st])
            olt = ldpool.tile([128, 64 * 5], F32, tag="ol")
            nc.sync.dma_start(out=olt[:], in_=ov[st])

            # repack into 32-col event slots (cols 23..31 zeroed)
            stg = stgpool.tile([128, 64 * 32], ACT_DT, tag="stg")
            s3 = stg[:].rearrange("p (e f) -> p e f", f=32)
            nc.vector.memset(s3[:, :, 23:32], 0.0)
            nc.vector.tensor_copy(
                s3[:, :, 0:18],
                xl[:].rearrange("p (e f) -> p e f", f=18))
            nc.vector.tensor_copy(
                s3[:, :, 18:23],
                olt[:].rearrange("p (e f) -> p e f", f=5))

            D_ps = dpool.tile([128, NCOL], F32, tag="dps")
            drive([pass_gen(stg, D_ps, 0), pass_gen(stg, D_ps, 1), carry])
            carry = None
            drive([pass_gen(stg, D_ps, 2), pass_gen(stg, D_ps, 3)])
            carry = head_gen(D_ps, st)
        drive([carry])

    nc.compile()
    return nc


class TileCtx:
    """TileContext + ExitStack in one `with`."""

    def __init__(self, nc):
        self.nc = nc

    def __enter__(self):
        self.ctx = ExitStack()
        self.tc = tile.TileContext(self.nc)
        self.tc.__enter__()
        return self.tc, self.ctx

    def __exit__(self, *a):
        self.ctx.close()
        return self.tc.__exit__(*a)


_CACHE = {}


def _get_program(wslices_key, wcols, bcols, nst, wslices, bslices):
    key = (wcols, bcols, nst)
    if key not in _CACHE:
        _CACHE[key] = _build_program(wslices, bslices, wcols, bcols, nst)
    return _CACHE[key]


def kernel(x, o, hl0_W, hl0_b, hl_W, hl_b, g0_W, g0_b, gp_W, gp_b,
           gf_W, gf_b, fc0_W, fc0_b, fc1_W, fc1_b, _nst=NST, _trace=False,
           _tmpdir=None):
    wp, bp = _prep_consts(hl0_W, hl0_b, hl_W, hl_b, g0_W, g0_b, gp_W, gp_b,
                          gf_W, gf_b, fc0_W, fc0_b, fc1_W, fc1_b)
    wpack = wp.array()
    assert wpack.dtype == ACT_NP
    bpack = bp.array()
    nc = _get_program(None, wpack.shape[1], bpack.shape[1], _nst,
                      wp.slices, bp.slices)

    bc = _nst * ST_EV
    ncores_used = min(NCORES, (np.asarray(x).shape[0] + bc - 1) // bc)
    x = np.ascontiguousarray(np.asarray(x, np.float32)).reshape(-1, 18)
    o = np.ascontiguousarray(np.asarray(o, np.float32)).reshape(-1, 5)

    in_maps = []
    for c in range(ncores_used):
        in_maps.append({
            "x": x[c * bc:(c + 1) * bc],
            "o": o[c * bc:(c + 1) * bc],
            "wpack": wpack,
            "bpack": bpack,
        })
    res = run_bass_kernel_spmd(nc, in_maps, list(range(ncores_used)),
                               trace=_trace, tmpdir=_tmpdir)
    ys = [res.results[c]["y"] for c in range(ncores_used)]
    out = np.concatenate(ys, axis=0)
    if _trace:
        kernel._last_exec_ns = res.exec_time_ns
    return out


kernel._last_exec_ns = None
